# revision 10
# baseline (speedup 1.0000x reference)
"""Trainium2 Bass kernel for an LSTM decoder with attention + greedy decode.

Model (per step t, 32 steps, batch 64):
  x = emb[tok]                         # [B, 512]
  4-layer LSTM (HID=256, PyTorch gate order i,f,g,o)
  dot-product attention over enc_h [128, B, 256]
  logits = sigmoid([h_top, ctx] @ w_fc.T + b_fc)   # [B, 32000]
  prob = softmax(logits); tok = argmax(prob)

Sharding over 8 NeuronCores:
  - LSTM replicated on every core (weight-load bound; sharding doesn't help)
  - attention batch-sharded (8 batch rows per core) + tiny ctx AllGather
  - FC vocab-sharded (4000 rows per core), per-step argmax resolved with an
    AllGather of per-core (max, argmax); softmax denominator via AllReduce
  - probs output written sharded, reassembled on host

All matmul paths are fp32: the top-2 logit gap distribution has min ~8e-6,
so any lower-precision matmul flips greedy argmaxes and diverges the decode.

Internal layouts are transposed ("unit on partitions, batch on free"):
  hT/cT per layer: [128, (2 hchunk, 64 b)]
  gates psum:      [128, (8 gate-ptile, 64 b)], gate ptile order i0i1f0f1o0o1g0g1
  FC:              z[b, vocab] via lhsT = hcT chunks, rhs = w_fcT
"""
import numpy as np

VOCAB, EMB, HID, BATCH, SRC = 32000, 512, 256, 64, 128
NCORE = 8
BL = BATCH // NCORE          # attention batch rows per core
VSH = VOCAB // NCORE         # vocab rows per core
GP = 8                       # gate ptiles = 4*HID/128
HC = HID // 128              # h chunks
XC = EMB // 128              # x chunks
NQ = 4                       # FC quarter passes
QW = 1024                    # z columns per quarter (2 banks x 512)

_CACHE = {}


def _build(T):
    import concourse.bass as bass
    import concourse.mybir as mybir
    import concourse.tile as tile
    from concourse import bacc
    from concourse.masks import make_identity

    F32 = mybir.dt.float32
    U32 = mybir.dt.uint32
    I32 = mybir.dt.int32
    AF = mybir.ActivationFunctionType
    OP = mybir.AluOpType

    nc = bacc.Bacc("TRN2", target_bir_lowering=False, debug=False,
                   num_devices=NCORE)

    # ---- DRAM parameters (per-core in_maps supply these) ----
    emb = nc.declare_dram_parameter("emb", [VOCAB, EMB], F32, isOutput=False)
    w0x = nc.declare_dram_parameter("w0x", [EMB, 4 * HID], F32, isOutput=False)
    w0h = nc.declare_dram_parameter("w0h", [HID, 4 * HID], F32, isOutput=False)
    wlx = [nc.declare_dram_parameter(f"wx{l}", [HID, 4 * HID], F32, isOutput=False)
           for l in (1, 2, 3)]
    wlh = [nc.declare_dram_parameter(f"wh{l}", [HID, 4 * HID], F32, isOutput=False)
           for l in (1, 2, 3)]
    encT_d = nc.declare_dram_parameter("encT", [HID, BL, SRC], F32, isOutput=False)
    encS_d = nc.declare_dram_parameter("encS", [SRC, BL, HID], F32, isOutput=False)
    wfc_d = nc.declare_dram_parameter("wfc", [2 * HID, VSH], F32, isOutput=False)
    bsel_d = nc.declare_dram_parameter("bsel", [BATCH, BL], F32, isOutput=False)
    tok0_d = nc.declare_dram_parameter("tok0", [BATCH, 1], I32, isOutput=False)
    coff_d = nc.declare_dram_parameter("coff", [BATCH, 1], F32, isOutput=False)
    probs_d = nc.declare_dram_parameter("probs", [T, BATCH, VSH], F32, isOutput=True)

    with tile.TileContext(nc) as tc:
        with (
            tc.tile_pool(name="const", bufs=1) as cp,
            tc.tile_pool(name="state", bufs=1) as st,
            tc.tile_pool(name="zp", bufs=1) as zp,
            tc.tile_pool(name="work", bufs=2) as wp,
            tc.tile_pool(name="ps_g", bufs=1, space="PSUM") as ps_g,
            tc.tile_pool(name="ps_a", bufs=2, space="PSUM") as ps_a,
            tc.tile_pool(name="ps_z", bufs=2, space="PSUM") as ps_z,
            tc.tile_pool(name="dram", bufs=2, space="DRAM") as dp,
        ):
            # ---- load constants ----
            w0x_s = cp.tile([128, XC, GP, 128], F32, tag="w0x")
            nc.sync.dma_start(w0x_s[:], w0x.rearrange("(c p) (g m) -> p c g m", p=128, m=128))
            w0h_s = cp.tile([128, HC, GP, 128], F32, tag="w0h")
            nc.sync.dma_start(w0h_s[:], w0h.rearrange("(c p) (g m) -> p c g m", p=128, m=128))
            wx_s, wh_s = [], []
            for i in range(3):
                tx = cp.tile([128, HC, GP, 128], F32, tag=f"wx{i}")
                nc.sync.dma_start(tx[:], wlx[i].rearrange("(c p) (g m) -> p c g m", p=128, m=128))
                th = cp.tile([128, HC, GP, 128], F32, tag=f"wh{i}")
                nc.sync.dma_start(th[:], wlh[i].rearrange("(c p) (g m) -> p c g m", p=128, m=128))
                wx_s.append(tx)
                wh_s.append(th)
            encT = cp.tile([128, HC, BL, SRC], F32, tag="encT")
            nc.sync.dma_start(encT[:], encT_d.rearrange("(c p) b s -> p c b s", p=128))
            encS = cp.tile([128, BL, HC, 128], F32, tag="encS")
            nc.sync.dma_start(encS[:], encS_d.rearrange("s b (c m) -> s b c m", m=128))
            wfc = cp.tile([128, 4, VSH], F32, tag="wfc")
            nc.sync.dma_start(wfc[:], wfc_d.rearrange("(c p) v -> p c v", p=128))
            coff = cp.tile([BATCH, 1], F32, tag="coff")
            nc.sync.dma_start(coff[:], coff_d[:, :])
            bsel = cp.tile([BATCH, BL], F32, tag="bsel")
            nc.sync.dma_start(bsel[:], bsel_d[:, :])
            ident = cp.tile([128, 128], F32, tag="ident")
            make_identity(nc, ident[:, :])

            # ---- state ----
            hT = [st.tile([128, HC, BATCH], F32, tag=f"h{l}", name=f"h{l}") for l in range(4)]
            cT = [st.tile([128, HC, BATCH], F32, tag=f"c{l}", name=f"c{l}") for l in range(4)]
            for t_ in hT + cT:
                nc.vector.memset(t_[:], 0.0)
            idx = st.tile([BATCH, 1], U32, tag="idx")
            nc.sync.dma_start(idx[:, :], tok0_d[:, :].bitcast(U32))

            for t in range(T):
                # ---- embedding gather + transpose ----
                xg = wp.tile([BATCH, EMB], F32, tag="xg")
                nc.gpsimd.indirect_dma_start(
                    out=xg[:, :], out_offset=None, in_=emb[:, :],
                    in_offset=bass.IndirectOffsetOnAxis(ap=idx[:, :1], axis=0))
                px = ps_a.tile([128, XC, BATCH], F32, tag="att")
                for c in range(XC):
                    nc.tensor.transpose(px[:, c, :], xg[:, c * 128:(c + 1) * 128],
                                        ident[0:BATCH, 0:BATCH])
                xT = wp.tile([128, XC, BATCH], F32, tag="xT")
                nc.vector.tensor_copy(xT[:], px[:])

                # ---- LSTM layers ----
                for l in range(4):
                    if l == 0:
                        ins = [(w0x_s, xT, XC), (w0h_s, hT[0], HC)]
                    else:
                        ins = [(wx_s[l - 1], hT[l - 1], HC), (wh_s[l - 1], hT[l], HC)]
                    ntot = sum(e[2] for e in ins)
                    pg = ps_g.tile([128, GP, BATCH], F32, tag="pg")
                    for g in range(GP):
                        k = 0
                        for (wt, xt, nck) in ins:
                            for c in range(nck):
                                nc.tensor.matmul(pg[:, g, :], wt[:, c, g, :],
                                                 xt[:, c, :],
                                                 start=(k == 0), stop=(k == ntot - 1))
                                k += 1
                    sig = wp.tile([128, 6, BATCH], F32, tag="sig")
                    nc.scalar.activation(sig[:], pg[:, 0:6, :], AF.Sigmoid)
                    tg = wp.tile([128, HC, BATCH], F32, tag="tg")
                    nc.scalar.activation(tg[:], pg[:, 6:8, :], AF.Tanh)
                    t1 = wp.tile([128, HC, BATCH], F32, tag="t1")
                    nc.vector.tensor_tensor(out=t1[:], in0=sig[:, 2:4, :],
                                            in1=cT[l][:], op=OP.mult)
                    t2 = wp.tile([128, HC, BATCH], F32, tag="t2")
                    nc.vector.tensor_tensor(out=t2[:], in0=sig[:, 0:2, :],
                                            in1=tg[:], op=OP.mult)
                    nc.vector.tensor_tensor(out=cT[l][:], in0=t1[:], in1=t2[:],
                                            op=OP.add)
                    tc_ = wp.tile([128, HC, BATCH], F32, tag="tc")
                    nc.scalar.activation(tc_[:], cT[l][:], AF.Tanh)
                    nc.vector.tensor_tensor(out=hT[l][:], in0=sig[:, 4:6, :],
                                            in1=tc_[:], op=OP.mult)

                # ---- attention (local batch slice) ----
                # hLT = h3 columns for this core's batch rows, selected with a
                # per-core one-hot matrix (programs are SPMD-identical, so the
                # slice offset must come from data, not code).
                h3 = hT[3]
                hts = wp.tile([BATCH, HC, 128], F32, tag="hts")
                for c in range(HC):
                    pht = ps_a.tile([BATCH, 128], F32, tag="att")
                    nc.tensor.transpose(pht[:, :], h3[:, c, :], ident[:, :])
                    nc.vector.tensor_copy(hts[:, c, :], pht[:, :])
                phl = ps_a.tile([128, HC, BL], F32, tag="att")
                for c in range(HC):
                    nc.tensor.matmul(phl[:, c, :], hts[:, c, :], bsel[:, :],
                                     start=True, stop=True)
                hLT = wp.tile([128, HC, BL], F32, tag="hLT")
                nc.vector.tensor_copy(hLT[:], phl[:])
                psc = ps_a.tile([128, BL], F32, tag="att")
                for bl in range(BL):
                    for c in range(HC):
                        nc.tensor.matmul(psc[:, bl:bl + 1], encT[:, c, bl, :],
                                         hLT[:, c, bl:bl + 1],
                                         start=(c == 0), stop=(c == HC - 1))
                sco = wp.tile([128, BL], F32, tag="sco")
                nc.vector.tensor_copy(sco[:], psc[:])
                pst = ps_a.tile([BL, SRC], F32, tag="att")
                nc.tensor.transpose(pst[:, :], sco[:, :], ident[:, :])
                nmx = wp.tile([BL, 1], F32, tag="nmx")
                nc.vector.tensor_reduce(nmx[:, :], pst[:, :],
                                        axis=mybir.AxisListType.X, op=OP.max,
                                        negate=True)
                esb = wp.tile([BL, SRC], F32, tag="esb")
                asum = wp.tile([BL, 1], F32, tag="asum")
                nc.scalar.activation(esb[:, :], pst[:, :], AF.Exp,
                                     bias=nmx[:, 0:1], accum_out=asum[:, 0:1])
                rec = wp.tile([BL, 1], F32, tag="rec")
                nc.vector.reciprocal(rec[:, :], asum[:, :])
                asb = wp.tile([BL, SRC], F32, tag="asb")
                nc.vector.tensor_scalar_mul(asb[:, :], esb[:, :], rec[:, 0:1])
                pat = ps_a.tile([128, BL], F32, tag="att")
                nc.tensor.transpose(pat[:, :], asb[:, :], ident[0:BL, 0:BL])
                aT = wp.tile([128, BL], F32, tag="aT")
                nc.vector.tensor_copy(aT[:], pat[:])
                pcx = ps_a.tile([128, HC, BL], F32, tag="att")
                for bl in range(BL):
                    for c in range(HC):
                        nc.tensor.matmul(pcx[:, c, bl:bl + 1], encS[:, bl, c, :],
                                         aT[:, bl:bl + 1], start=True, stop=True)
                cxl = wp.tile([128, HC, BL], F32, tag="cxl")
                nc.vector.tensor_copy(cxl[:], pcx[:])

                # ctx allgather
                cxi = dp.tile([128, HC, BL], F32, tag="cxi")
                nc.sync.dma_start(cxi[:], cxl[:])
                cxo = dp.tile([NCORE * 128, HC, BL], F32, tag="cxo")
                nc.gpsimd.collective_compute(
                    "AllGather", OP.bypass,
                    replica_groups=[list(range(NCORE))],
                    ins=[cxi[:]], outs=[cxo[:]])
                ctxT = wp.tile([128, HC, NCORE, BL], F32, tag="ctxT")
                nc.sync.dma_start(ctxT[:], cxo.rearrange("(k p) c b -> p c k b", p=128))

                # ---- FC (vocab shard), quarter passes ----
                Z = zp.tile([BATCH, NQ * QW], F32, tag="Z")

                def lhs(c):
                    if c < HC:
                        return h3[:, c, :]
                    return ctxT[:, c - HC, :, :]

                for q in range(NQ):
                    zq = ps_z.tile([BATCH, 2, 512], F32, tag="zq")
                    for c in range(4):
                        for v2 in range(2):
                            v0 = q * QW + v2 * 512
                            w = min(512, VSH - v0)
                            nc.tensor.matmul(zq[:, v2, 0:w], lhs(c),
                                             wfc[:, c, v0:v0 + w],
                                             start=(c == 0), stop=(c == 3))
                    nc.vector.tensor_copy(Z[:, q * QW:(q + 1) * QW], zq[:])

                # ---- local argmax on pre-activations ----
                mx8 = wp.tile([BATCH, 8], F32, tag="mx8")
                ix8 = wp.tile([BATCH, 8], U32, tag="ix8")
                nc.vector.max_with_indices(mx8[:, :], ix8[:, :], Z[:, 0:VSH])
                stats = wp.tile([BATCH, 4], F32, tag="stats")
                nc.vector.tensor_copy(stats[:, 0:1], mx8[:, 0:1])
                idxf = wp.tile([BATCH, 1], F32, tag="idxf")
                nc.vector.tensor_copy(idxf[:, :], ix8[:, 0:1])
                nc.vector.tensor_tensor(out=stats[:, 1:2], in0=idxf[:, :],
                                        in1=coff[:, :], op=OP.add)

                if t < T - 1:
                    sti = dp.tile([BATCH, 4], F32, tag="sti")
                    nc.sync.dma_start(sti[:], stats[:])
                    sto = dp.tile([NCORE * BATCH, 4], F32, tag="sto")
                    nc.gpsimd.collective_compute(
                        "AllGather", OP.bypass,
                        replica_groups=[list(range(NCORE))],
                        ins=[sti[:]], outs=[sto[:]])
                    gsb = wp.tile([BATCH, NCORE, 4], F32, tag="gsb")
                    nc.sync.dma_start(gsb[:], sto.rearrange("(k b) w -> b k w", b=BATCH))
                    gmx = wp.tile([BATCH, 1], F32, tag="gmx")
                    nc.vector.tensor_reduce(gmx[:, :], gsb[:, :, 0],
                                            axis=mybir.AxisListType.X, op=OP.max)
                    eq = wp.tile([BATCH, NCORE], F32, tag="eq")
                    nc.vector.tensor_tensor(out=eq[:, :], in0=gsb[:, :, 0],
                                            in1=gmx[:, 0:1].to_broadcast([BATCH, NCORE]),
                                            op=OP.is_equal)
                    cand = wp.tile([BATCH, NCORE], F32, tag="cand")
                    nc.vector.tensor_tensor(out=cand[:, :], in0=eq[:, :],
                                            in1=gsb[:, :, 1], op=OP.mult)
                    tokf = wp.tile([BATCH, 1], F32, tag="tokf")
                    nc.vector.tensor_reduce(tokf[:, :], cand[:, :],
                                            axis=mybir.AxisListType.X, op=OP.max)
                    nc.vector.tensor_copy(idx[:, :], tokf[:, :])

                # ---- probs: sigmoid -> exp (+sum) -> allreduce -> scale ----
                nc.scalar.activation(Z[:, 0:VSH], Z[:, 0:VSH], AF.Sigmoid)
                dloc = wp.tile([BATCH, 1], F32, tag="dloc")
                nc.scalar.activation(Z[:, 0:VSH], Z[:, 0:VSH], AF.Exp,
                                     accum_out=dloc[:, 0:1])
                dni = dp.tile([BATCH, 1], F32, tag="dni")
                nc.sync.dma_start(dni[:], dloc[:])
                dno = dp.tile([BATCH, 1], F32, tag="dno")
                nc.gpsimd.collective_compute(
                    "AllReduce", OP.add,
                    replica_groups=[list(range(NCORE))],
                    ins=[dni[:]], outs=[dno[:]])
                den = wp.tile([BATCH, 1], F32, tag="den")
                nc.sync.dma_start(den[:], dno[:])
                drc = wp.tile([BATCH, 1], F32, tag="drc")
                nc.vector.reciprocal(drc[:, :], den[:, :])
                nc.vector.tensor_scalar_mul(Z[:, 0:VSH], Z[:, 0:VSH], drc[:, 0:1])
                nc.sync.dma_start(probs_d[t], Z[:, 0:VSH])

    nc.compile()
    return nc


def _prep_inputs(enc_h, emb, w_ih_l0, w_hh_l0, b_l0, w_ih_rest, w_hh_rest,
                 b_rest, w_fc, b_fc, start_code):
    """Build the 8 per-core input maps (numpy only)."""
    H = HID
    perm = np.concatenate([np.arange(0, H), np.arange(H, 2 * H),
                           np.arange(3 * H, 4 * H), np.arange(2 * H, 3 * H)])
    assert not np.any(b_l0) and not np.any(b_rest) and not np.any(b_fc), \
        "nonzero biases not supported by this kernel build"

    w0x = np.ascontiguousarray(w_ih_l0[perm, :].T)     # [512, 1024]
    w0h = np.ascontiguousarray(w_hh_l0[perm, :].T)     # [256, 1024]
    wx = [np.ascontiguousarray(w_ih_rest[i][perm, :].T) for i in range(3)]
    wh = [np.ascontiguousarray(w_hh_rest[i][perm, :].T) for i in range(3)]

    in_maps = []
    for k in range(NCORE):
        bs = slice(k * BL, (k + 1) * BL)
        E = enc_h[:, bs, :]                            # [128, 8, 256]
        m = {
            "emb": emb,
            "w0x": w0x, "w0h": w0h,
            "wx1": wx[0], "wh1": wh[0],
            "wx2": wx[1], "wh2": wh[1],
            "wx3": wx[2], "wh3": wh[2],
            "encT": np.ascontiguousarray(E.transpose(2, 1, 0)),  # [256, 8, 128]
            "encS": np.ascontiguousarray(E),                     # [128, 8, 256]
            "wfc": np.ascontiguousarray(w_fc[k * VSH:(k + 1) * VSH, :].T),
            "bsel": np.eye(BATCH, dtype=np.float32)[:, k * BL:(k + 1) * BL].copy(),
            "tok0": np.full((BATCH, 1), start_code, np.int32),
            "coff": np.full((BATCH, 1), float(k * VSH), np.float32),
        }
        in_maps.append(m)
    return in_maps


def kernel(enc_h, emb, w_ih_l0, w_hh_l0, b_l0, w_ih_rest, w_hh_rest, b_rest,
           w_fc, b_fc, max_sentence_len, start_code):
    from concourse.bass_utils import run_bass_kernel_spmd

    T = int(max_sentence_len)
    args = [np.asarray(np.float32(0) + a, np.float32) if np.asarray(a).dtype != np.float32
            else np.asarray(a) for a in
            (enc_h, emb, w_ih_l0, w_hh_l0, b_l0, w_ih_rest, w_hh_rest, b_rest,
             w_fc, b_fc)]
    in_maps = _prep_inputs(*args, int(start_code))

    if T not in _CACHE:
        _CACHE[T] = _build(T)
    nc = _CACHE[T]
    res = run_bass_kernel_spmd(nc, in_maps, core_ids=list(range(NCORE))).results

    out = np.empty((T, BATCH, VOCAB), np.float32)
    for k in range(NCORE):
        out[:, :, k * VSH:(k + 1) * VSH] = res[k]["probs"]
    return out
